# revision 13
# baseline (speedup 1.0000x reference)
"""Trainium2 Bass kernel for nn_MultiHeadAttention_824633721543.

MHA with periodic prefix mask: allowed iff (q % 256) >= (k % 256).
B=2, S=2048, D=768, H=12, Dk=64, WINDOW=256.

Sharding: 8 cores = 2 batches x 4 head-groups (3 heads each). Each core
computes q/k/v projections for its heads, the masked softmax attention, and
a partial O-projection; the host sums the 4 partials per batch and adds bo.

v3:
  - bf16 for x / Wq / Wk / Wv input side (halves input DMA + full-rate
    matmuls) and for probabilities/V on the P@V side (halves mask cost);
    scores themselves accumulate in fp32 PSUM from fp32r q/k tiles.
  - one global software-pipelined chain over all (head, q-group) units:
    scores for chunk t issue together with P@V for chunk t-2, across unit
    boundaries, so the tensor engine stream never waits on the
    exp -> mask chain and the HAM clock gate stays at 2.4 GHz.
  - normalization reuses the unit's dead PSUM accumulator for the
    reciprocal broadcast (no extra PSUM), final scale on GpSimd.
  - O-projection for the even-q half is interleaved into the odd-q pass
    as tensor-engine filler; its output DMA overlaps attention compute.
"""

import sys

sys.path.insert(0, "/opt/trn_rl_repo")

import numpy as np

B = 2
S = 2048
D = 768
DK = 64
WIN = 256
NW = S // WIN   # 8 windows
NHC = 3         # heads per core
DH = NHC * DK   # 192
NT = S // 128   # 16 q tiles
LAG = 2         # scores run this many chunks ahead of P@V on the PE

_CACHE = {}


def _build_program():
    import concourse.tile as tile
    from concourse import mybir, bacc
    from contextlib import ExitStack

    f32 = mybir.dt.float32
    f32r = mybir.dt.float32r
    bf16 = mybir.dt.bfloat16
    Exp = mybir.ActivationFunctionType.Exp
    mult = mybir.AluOpType.mult
    add = mybir.AluOpType.add

    nc = bacc.Bacc("TRN2", target_bir_lowering=False, debug=False)

    xT = nc.dram_tensor("xT", [D, S], bf16, kind="ExternalInput").ap()
    w1 = nc.dram_tensor("w1", [D, 256], bf16, kind="ExternalInput").ap()  # [qh0|qh1|kh0|kh1]
    w2 = nc.dram_tensor("w2", [D, 128], bf16, kind="ExternalInput").ap()  # [qh2|kh2]
    wv = nc.dram_tensor("wv", [D, 192], bf16, kind="ExternalInput").ap()  # WvT
    wo = nc.dram_tensor("wo", [DH, D], f32r, kind="ExternalInput").ap()
    btA = nc.dram_tensor("btA", [128, 1], f32, kind="ExternalInput").ap()
    btB = nc.dram_tensor("btB", [128, 1], f32, kind="ExternalInput").ap()
    btC = nc.dram_tensor("btC", [64, 1], f32, kind="ExternalInput").ap()
    btD = nc.dram_tensor("btD", [64, 1], f32, kind="ExternalInput").ap()
    bvb = nc.dram_tensor("bvb", [128, 192], f32, kind="ExternalInput").ap()
    triu = nc.dram_tensor("triu", [128, 128], bf16, kind="ExternalInput").ap()
    onesd = nc.dram_tensor("onesd", [128, 64], f32r, kind="ExternalInput").ap()
    out = nc.dram_tensor("out", [S, D], f32, kind="ExternalOutput").ap()

    with tile.TileContext(nc) as tc, ExitStack() as ctx:
        consts = ctx.enter_context(tc.tile_pool(name="consts", bufs=1))
        qkv = ctx.enter_context(tc.tile_pool(name="qkv", bufs=1))
        pt_pool = ctx.enter_context(tc.tile_pool(name="pt", bufs=4))
        nrm_pool = ctx.enter_context(tc.tile_pool(name="nrm", bufs=2))
        ost_pool = ctx.enter_context(tc.tile_pool(name="ost", bufs=2))

        # ---- const tiles ----
        btA_sb = consts.tile([128, 1], f32, tag="btA")
        btB_sb = consts.tile([128, 1], f32, tag="btB")
        btC_sb = consts.tile([64, 1], f32, tag="btC")
        btD_sb = consts.tile([64, 1], f32, tag="btD")
        bvb_sb = consts.tile([128, 192], f32, tag="bvb")
        triu_sb = consts.tile([128, 128], bf16, tag="triu")
        ones_row = consts.tile([128, 64], f32r, tag="ones_row")
        scratch = consts.tile([1, 8], f32, tag="scratch")
        w1_sb = [consts.tile([128, 256], bf16, tag=f"w1_{k}", name=f"w1s{k}")
                 for k in range(6)]
        w2_sb = [consts.tile([128, 128], bf16, tag=f"w2_{k}", name=f"w2s{k}")
                 for k in range(6)]
        wv_sb = [consts.tile([128, 192], bf16, tag=f"wv_{k}", name=f"wvs{k}")
                 for k in range(6)]
        wo_sb = [consts.tile([64, D], f32r, tag=f"wo{h}", name=f"wos{h}")
                 for h in range(NHC)]

        # preload the exp table set while DMAs run
        nc.scalar.memzero(scratch)
        nc.scalar.activation(out=scratch, in_=scratch, func=Exp)

        xtp_cm = tc.tile_pool(name="xtp", bufs=1)
        xtp = xtp_cm.__enter__()
        # x^T in 12 [128, 1024] bf16 tiles (k-slice x column half)
        xts = [[xtp.tile([128, 1024], bf16, tag=f"xt{k}_{m}", name=f"xt{k}_{m}")
                for m in range(2)] for k in range(6)]

        # ---- DMA emission order == arrival priority ----
        nc.sync.dma_start(out=btA_sb, in_=btA)
        nc.sync.dma_start(out=btB_sb, in_=btB)
        nc.sync.dma_start(out=btC_sb, in_=btC)
        nc.sync.dma_start(out=btD_sb, in_=btD)
        nc.sync.dma_start(out=bvb_sb, in_=bvb)
        nc.sync.dma_start(out=triu_sb, in_=triu)
        nc.sync.dma_start(out=ones_row, in_=onesd)
        for k in range(6):
            nc.sync.dma_start(out=w1_sb[k], in_=w1[k * 128:(k + 1) * 128, :])
            nc.sync.dma_start(out=w2_sb[k], in_=w2[k * 128:(k + 1) * 128, :])
            nc.sync.dma_start(out=wv_sb[k], in_=wv[k * 128:(k + 1) * 128, :])
            nc.sync.dma_start(out=xts[k][0],
                              in_=xT[k * 128:(k + 1) * 128, 0:1024])
        for k in range(6):
            nc.sync.dma_start(out=xts[k][1],
                              in_=xT[k * 128:(k + 1) * 128, 1024:2048])
        for h in range(NHC):
            nc.sync.dma_start(out=wo_sb[h], in_=wo[64 * h:64 * (h + 1), :])

        # ---- long-lived activation tiles ----
        tileA = qkv.tile([128, S], f32r, tag="tileA")  # [qT_h0|qT_h1], q-permuted
        tileB = qkv.tile([128, S], f32r, tag="tileB")  # [kT_h0|kT_h1], natural
        tileC = qkv.tile([64, S], f32r, tag="tileC")   # qT_h2, permuted
        tileD = qkv.tile([64, S], f32r, tag="tileD")   # kT_h2, natural
        v_sb = [qkv.tile([128, 196], bf16, tag=f"v{i}", name=f"vsb{i}")
                for i in range(NT)]
        attnT = [qkv.tile([64, S], f32r, tag=f"attnT{h}", name=f"attnT{h}")
                 for h in range(NHC)]

        def permuted_copy(dst, rows, ps, n, bias):
            """psum 512-span n -> dst cols with even/odd tile permutation."""
            pr3 = ps[0:rows, :].rearrange("p (c two k) -> p c two k", two=2, k=128)
            dr = dst[0:rows, :]
            nc.vector.tensor_scalar_add(
                out=dr[:, 256 * n:256 * n + 256].rearrange("p (c k) -> p c k", k=128),
                in0=pr3[:, :, 0, :], scalar1=bias[0:rows, :])
            nc.vector.tensor_scalar_add(
                out=dr[:, 1024 + 256 * n:1024 + 256 * n + 256].rearrange(
                    "p (c k) -> p c k", k=128),
                in0=pr3[:, :, 1, :], scalar1=bias[0:rows, :])

        # ---- stage A: projections, V interleaved per span ----
        with tc.tile_pool(name="psA", bufs=2, space="PSUM") as psA, \
             tc.tile_pool(name="psV", bufs=2, space="PSUM") as psV:
            for n in range(4):
                xn = [xts[k][n // 2][:, 512 * (n % 2):512 * (n % 2) + 512]
                      for k in range(6)]
                psa = psA.tile([128, 512], f32, tag="psA")
                for k in range(6):
                    nc.tensor.matmul(psa, w1_sb[k][:, 0:128], xn[k],
                                     start=(k == 0), stop=(k == 5))
                permuted_copy(tileA, 128, psa, n, btA_sb)
                psb = psA.tile([128, 512], f32, tag="psA")
                for k in range(6):
                    nc.tensor.matmul(psb, w1_sb[k][:, 128:256], xn[k],
                                     start=(k == 0), stop=(k == 5))
                nc.vector.tensor_scalar_add(
                    out=tileB[:, 512 * n:512 * (n + 1)], in0=psb, scalar1=btB_sb)
                psq = psA.tile([64, 512], f32, tag="psq")
                psk = psA.tile([64, 512], f32, tag="psq")
                for k in range(6):
                    nc.tensor.matmul(psq, w2_sb[k][:, 0:64], xn[k],
                                     start=(k == 0), stop=(k == 5))
                    nc.tensor.matmul(psk, w2_sb[k][:, 64:128], xn[k],
                                     start=(k == 0), stop=(k == 5))
                permuted_copy(tileC, 64, psq, n, btC_sb)
                nc.vector.tensor_scalar_add(
                    out=tileD[:, 512 * n:512 * (n + 1)], in0=psk, scalar1=btD_sb)
                # V projection for the 4 s-tiles of this span
                for j in range(4):
                    st = 4 * n + j
                    psv = psV.tile([128, 192], f32, tag="psv")
                    for k in range(6):
                        nc.tensor.matmul(
                            psv, xn[k][:, 128 * j:128 * (j + 1)],
                            wv_sb[k], start=(k == 0), stop=(k == 5))
                    vt = v_sb[st]
                    nc.vector.tensor_tensor(
                        out=vt[:, 0:195].rearrange("p (h c) -> p h c", c=65)[:, :, 0:64],
                        in0=psv.rearrange("p (h c) -> p h c", c=64),
                        in1=bvb_sb.rearrange("p (h c) -> p h c", c=64), op=add)
                    nc.vector.tensor_copy(
                        out=vt[:, 0:195].rearrange("p (h c) -> p h c", c=65)[:, :, 64:65],
                        in_=ones_row[:, 0:3].unsqueeze(2))

        xtp_cm.__exit__(None, None, None)

        # ---- stage B + C: one global pipelined chain ----
        heads = [
            dict(q=(tileA, 0), k=(tileB, 0)),
            dict(q=(tileA, 64), k=(tileB, 64)),
            dict(q=(tileC, 0), k=(tileD, 0)),
        ]
        triu_b = triu_sb.unsqueeze(1).broadcast_to([128, 8, 128])

        units = [(0, 0), (1, 0), (2, 0), (0, 1), (1, 1), (2, 1)]
        flat = []  # (ui, h, grp, w, half, masked, c, C)
        for ui, (h, grp) in enumerate(units):
            if grp == 0:
                chs = [(w, 0, True) for w in range(NW)]
            else:
                chs = [(w, half, half == 1) for w in range(NW) for half in (0, 1)]
            for c, (w, half, masked) in enumerate(chs):
                flat.append((ui, h, grp, w, half, masked, c, len(chs)))
        n_flat = len(flat)  # 72

        with tc.tile_pool(name="sc", bufs=2, space="PSUM") as sc_pool, \
             tc.tile_pool(name="po", bufs=2, space="PSUM") as po_pool:

            po_of = {}
            first_of = {}
            raw_of = {}
            pts = {}
            inject = {}

            def add_inject(slot, fn):
                inject.setdefault(slot, []).append(fn)

            def emit_scores(t):
                ui, h, grp, w, half, masked, c, C = flat[t]
                if c == 0:
                    po_of[ui] = po_pool.tile([128, 1024], f32, tag="po",
                                             name=f"po{ui}")
                    first_of[ui] = [True, True]
                qt, qoff = heads[h]["q"]
                kt, koff = heads[h]["k"]
                qv = qt[qoff:qoff + 64, 1024 * grp:1024 * (grp + 1)]
                kv = kt[koff:koff + 64, :]
                kblk = kv[:, WIN * w + 128 * half:WIN * w + 128 * half + 128]
                sc = sc_pool.tile([128, 1024], f32, tag="sc")
                for sub in range(2):
                    nc.tensor.matmul(
                        sc[:, 512 * sub:512 * (sub + 1)], kblk,
                        qv[:, 512 * sub:512 * (sub + 1)],
                        start=True, stop=True)
                pt = pt_pool.tile([128, 1024], bf16, tag="pt")
                nc.scalar.activation(out=pt, in_=sc, func=Exp, scale=0.125)
                if masked:
                    p3 = pt.rearrange("p (c k) -> p c k", k=128)
                    nc.vector.tensor_mul(out=p3, in0=p3, in1=triu_b)
                pts[t] = pt

            def emit_pv(t):
                """Returns ui if this was the unit's last chunk."""
                ui, h, grp, w, half, masked, c, C = flat[t]
                po = po_of[ui]
                first = first_of[ui]
                vsl = v_sb[2 * w + half][:, 65 * h:65 * h + 65]
                pt = pts.pop(t)
                for sub in range(2):
                    nc.tensor.matmul(
                        po[0:65, 512 * sub:512 * (sub + 1)],
                        vsl, pt[:, 512 * sub:512 * (sub + 1)],
                        start=first[sub], stop=(c == C - 1))
                    first[sub] = False
                return ui if c == C - 1 else None

            def make_norm(ui, sliced=False):
                h, grp = units[ui]
                raw = raw_of[ui]

                def emit_norm():
                    rec_ps = sc_pool.tile([128, 1024], f32, tag="sc")
                    for sub in range(2):
                        nc.tensor.matmul(
                            rec_ps[0:64, 512 * sub:512 * (sub + 1)],
                            ones_row[64:65, :],
                            raw[64:65, 512 * sub:512 * (sub + 1)],
                            start=True, stop=True)
                    rec = nrm_pool.tile([64, 1024], f32, tag="rec")
                    nc.vector.reciprocal_approx_fast(out=rec, in_=rec_ps[0:64, :])
                    if not sliced:
                        nc.gpsimd.tensor_tensor(
                            out=attnT[h][:, 1024 * grp:1024 * (grp + 1)],
                            in0=raw[0:64, :], in1=rec, op=mult)
                    else:
                        # per-128-col slices, each unblocking one O-proj tile
                        for i in range(8):
                            nc.vector.tensor_tensor(
                                out=attnT[h][:, 1024 * grp + 128 * i:
                                             1024 * grp + 128 * (i + 1)],
                                in0=raw[0:64, 128 * i:128 * (i + 1)],
                                in1=rec[:, 128 * i:128 * (i + 1)], op=mult)
                            emit_stage_c(8 * grp + i)
                return emit_norm

            def emit_stage_c(p):
                pso = sc_pool.tile([128, D], f32, tag="sc")
                for (n0, n1) in ((0, 512), (512, 768)):
                    for h in range(NHC):
                        nc.tensor.matmul(
                            pso[:, n0:n1],
                            attnT[h][:, 128 * p:128 * (p + 1)],
                            wo_sb[h][:, n0:n1],
                            start=(h == 0), stop=(h == NHC - 1))
                ot = ost_pool.tile([128, D], f32, tag="ot")
                nc.vector.tensor_copy(out=ot, in_=pso)
                t = 2 * p if p < 8 else 2 * (p - 8) + 1
                nc.sync.dma_start(out=out[128 * t:128 * (t + 1), :], in_=ot)

            for s in range(n_flat + LAG):
                if s < n_flat:
                    emit_scores(s)
                for fn in inject.pop(s, []):
                    fn()
                if s >= LAG:
                    ui_done = emit_pv(s - LAG)
                    if ui_done is not None:
                        raw = nrm_pool.tile([65, 1024], f32r, tag="raw")
                        nc.vector.tensor_copy(out=raw, in_=po_of[ui_done][0:65, :])
                        raw_of[ui_done] = raw
                        add_inject(s + 2, make_norm(ui_done,
                                                    sliced=(ui_done == 5)))
                        if ui_done == 2:
                            # grp0 O-proj tiles as PE filler through grp1
                            for i in range(8):
                                add_inject(s + 4 + 5 * i,
                                           (lambda p: lambda: emit_stage_c(p))(i))
            for slot in sorted(inject):
                for fn in inject[slot]:
                    fn()

    nc.compile()
    return nc


def _prep_core_inputs(inputs, c):
    import ml_dtypes
    bf16 = ml_dtypes.bfloat16
    x = inputs["x"]
    Wq, bq = inputs["Wq"], inputs["bq"]
    Wk, bk = inputs["Wk"], inputs["bk"]
    Wv, bv = inputs["Wv"], inputs["bv"]
    Wo = inputs["Wo"]
    b = c // 4
    r0 = (c % 4) * DH  # first feature row of this core's 192-row head block

    xT = np.ascontiguousarray(np.asarray(x[b]).T.astype(bf16))
    W1 = np.ascontiguousarray(np.concatenate(
        [Wq[r0:r0 + 128].T, Wk[r0:r0 + 128].T], axis=1).astype(bf16))
    W2 = np.ascontiguousarray(np.concatenate(
        [Wq[r0 + 128:r0 + 192].T, Wk[r0 + 128:r0 + 192].T], axis=1).astype(bf16))
    Wvp = np.ascontiguousarray(Wv[r0:r0 + 192].T.astype(bf16))
    wo = np.ascontiguousarray(Wo[:, r0:r0 + 192].T.astype(np.float32))

    return dict(
        xT=xT, w1=W1, w2=W2, wv=Wvp, wo=wo,
        btA=np.ascontiguousarray(bq[r0:r0 + 128].reshape(128, 1).astype(np.float32)),
        btB=np.ascontiguousarray(bk[r0:r0 + 128].reshape(128, 1).astype(np.float32)),
        btC=np.ascontiguousarray(bq[r0 + 128:r0 + 192].reshape(64, 1).astype(np.float32)),
        btD=np.ascontiguousarray(bk[r0 + 128:r0 + 192].reshape(64, 1).astype(np.float32)),
        bvb=np.ascontiguousarray(np.tile(
            bv[r0:r0 + 192].reshape(1, 192), (128, 1)).astype(np.float32)),
        triu=np.ascontiguousarray(np.triu(np.ones((128, 128))).astype(bf16)),
        onesd=np.ones((128, 64), np.float32),
    )


def _install_ntff_hook():
    """Register antenv.axon_hooks with a ctypes NTFF profile hook so
    run_bass_kernel_spmd(trace=True) can capture device-side exec time."""
    import types, ctypes, contextlib, importlib

    try:
        import antenv.axon_hooks  # noqa: F401
        return
    except ImportError:
        pass
    so_path = "/opt/axon/libaxon_pjrt.so"
    lib = ctypes.CDLL(so_path)
    if not hasattr(lib, "axon_start_nrt_profile"):
        return
    lib.axon_start_nrt_profile.argtypes = [
        ctypes.POINTER(ctypes.c_int64), ctypes.c_size_t]
    lib.axon_start_nrt_profile.restype = ctypes.c_int64
    lib.axon_stop_nrt_profile.argtypes = [ctypes.c_char_p]
    lib.axon_stop_nrt_profile.restype = ctypes.c_int64

    @contextlib.contextmanager
    def _hook(output_dir, device_ids):
        import jax
        jax.devices()
        if device_ids:
            ids = (ctypes.c_int64 * len(device_ids))(*device_ids)
            rc = lib.axon_start_nrt_profile(ids, len(device_ids))
        else:
            rc = lib.axon_start_nrt_profile(None, 0)
        if rc != 0:
            raise RuntimeError(f"axon_start_nrt_profile rc={rc}")
        try:
            yield
        finally:
            n = lib.axon_stop_nrt_profile(str(output_dir).encode())
            print(f"profile: {n} file(s) written to {output_dir}")

    mod = types.ModuleType("antenv.axon_hooks")
    mod.get_axon_ntff_profile_hook = lambda: _hook
    mod.set_axon_ntff_profile_hook = lambda h: None
    sys.modules["antenv.axon_hooks"] = mod
    import antenv
    antenv.axon_hooks = mod


def kernel(**inputs):
    import os
    from concourse import bass_utils

    if "nc" not in _CACHE:
        _CACHE["nc"] = _build_program()
    nc = _CACHE["nc"]

    trace = bool(os.environ.get("MHA_TRACE"))
    kwargs = {}
    if trace:
        import shutil
        _install_ntff_hook()
        kwargs = dict(trace=True, tmpdir="/tmp/mha_trace")
        shutil.rmtree("/tmp/mha_trace", ignore_errors=True)
        os.makedirs("/tmp/mha_trace", exist_ok=True)

    in_maps = [_prep_core_inputs(inputs, c) for c in range(8)]
    res = bass_utils.run_bass_kernel_spmd(
        nc, in_maps, core_ids=list(range(8)), **kwargs)
    _CACHE["last_results"] = res
    if trace and res.exec_time_ns is not None:
        print(f"HW exec time: {res.exec_time_ns} ns")
    out = np.zeros((B, S, D), np.float32)
    for c in range(8):
        out[c // 4] += res.results[c]["out"]
    out += np.asarray(inputs["bo"], np.float32).reshape(1, 1, D)
    return out


# revision 14
# speedup vs baseline: 1.1403x; 1.1403x over previous
"""Trainium2 Bass kernel for nn_MultiHeadAttention_824633721543.

MHA with periodic prefix mask: allowed iff (q % 256) >= (k % 256).
B=2, S=2048, D=768, H=12, Dk=64, WINDOW=256.

Sharding: 8 cores = 2 batches x 4 head-groups (3 heads each). Each core
computes q/k/v projections for its heads, the masked softmax attention, and
a partial O-projection; the host sums the 4 partials per batch and adds bo.

v3:
  - bf16 for x / Wq / Wk / Wv input side (halves input DMA + full-rate
    matmuls) and for probabilities/V on the P@V side (halves mask cost);
    scores themselves accumulate in fp32 PSUM from fp32r q/k tiles.
  - one global software-pipelined chain over all (head, q-group) units:
    scores for chunk t issue together with P@V for chunk t-2, across unit
    boundaries, so the tensor engine stream never waits on the
    exp -> mask chain and the HAM clock gate stays at 2.4 GHz.
  - normalization reuses the unit's dead PSUM accumulator for the
    reciprocal broadcast (no extra PSUM), final scale on GpSimd.
  - O-projection for the even-q half is interleaved into the odd-q pass
    as tensor-engine filler; its output DMA overlaps attention compute.
"""

import sys

sys.path.insert(0, "/opt/trn_rl_repo")

import numpy as np

B = 2
S = 2048
D = 768
DK = 64
WIN = 256
NW = S // WIN   # 8 windows
NHC = 3         # heads per core
DH = NHC * DK   # 192
NT = S // 128   # 16 q tiles
LAG = 2         # scores run this many chunks ahead of P@V on the PE

_CACHE = {}


def _build_program():
    import concourse.tile as tile
    from concourse import mybir, bacc
    from contextlib import ExitStack

    f32 = mybir.dt.float32
    f32r = mybir.dt.float32r
    bf16 = mybir.dt.bfloat16
    Exp = mybir.ActivationFunctionType.Exp
    mult = mybir.AluOpType.mult
    add = mybir.AluOpType.add

    nc = bacc.Bacc("TRN2", target_bir_lowering=False, debug=False)

    xT = nc.dram_tensor("xT", [D, S], bf16, kind="ExternalInput").ap()
    w1 = nc.dram_tensor("w1", [D, 256], bf16, kind="ExternalInput").ap()  # [qh0|qh1|kh0|kh1]
    w2 = nc.dram_tensor("w2", [D, 128], bf16, kind="ExternalInput").ap()  # [qh2|kh2]
    wv = nc.dram_tensor("wv", [D, 192], bf16, kind="ExternalInput").ap()  # WvT
    wo = nc.dram_tensor("wo", [DH, D], f32r, kind="ExternalInput").ap()
    btA = nc.dram_tensor("btA", [128, 1], f32, kind="ExternalInput").ap()
    btB = nc.dram_tensor("btB", [128, 1], f32, kind="ExternalInput").ap()
    btC = nc.dram_tensor("btC", [64, 1], f32, kind="ExternalInput").ap()
    btD = nc.dram_tensor("btD", [64, 1], f32, kind="ExternalInput").ap()
    bvb = nc.dram_tensor("bvb", [128, 192], f32, kind="ExternalInput").ap()
    triu = nc.dram_tensor("triu", [128, 128], bf16, kind="ExternalInput").ap()
    onesd = nc.dram_tensor("onesd", [128, 64], f32r, kind="ExternalInput").ap()
    out = nc.dram_tensor("out", [S, D], f32, kind="ExternalOutput").ap()

    with tile.TileContext(nc) as tc, ExitStack() as ctx:
        consts = ctx.enter_context(tc.tile_pool(name="consts", bufs=1))
        qkv = ctx.enter_context(tc.tile_pool(name="qkv", bufs=1))
        pt_pool = ctx.enter_context(tc.tile_pool(name="pt", bufs=4))
        nrm_pool = ctx.enter_context(tc.tile_pool(name="nrm", bufs=2))
        ost_pool = ctx.enter_context(tc.tile_pool(name="ost", bufs=2))

        # ---- const tiles ----
        btA_sb = consts.tile([128, 1], f32, tag="btA")
        btB_sb = consts.tile([128, 1], f32, tag="btB")
        btC_sb = consts.tile([64, 1], f32, tag="btC")
        btD_sb = consts.tile([64, 1], f32, tag="btD")
        bvb_sb = consts.tile([128, 192], f32, tag="bvb")
        triu_sb = consts.tile([128, 128], bf16, tag="triu")
        ones_row = consts.tile([128, 64], f32r, tag="ones_row")
        scratch = consts.tile([1, 8], f32, tag="scratch")
        w1_sb = [consts.tile([128, 256], bf16, tag=f"w1_{k}", name=f"w1s{k}")
                 for k in range(6)]
        w2_sb = [consts.tile([128, 128], bf16, tag=f"w2_{k}", name=f"w2s{k}")
                 for k in range(6)]
        wv_sb = [consts.tile([128, 192], bf16, tag=f"wv_{k}", name=f"wvs{k}")
                 for k in range(6)]
        wo_sb = [consts.tile([64, D], f32r, tag=f"wo{h}", name=f"wos{h}")
                 for h in range(NHC)]

        # preload the exp table set while DMAs run
        nc.scalar.memzero(scratch)
        nc.scalar.activation(out=scratch, in_=scratch, func=Exp)

        xtp_cm = tc.tile_pool(name="xtp", bufs=1)
        xtp = xtp_cm.__enter__()
        # x^T in 12 [128, 1024] bf16 tiles (k-slice x column half)
        xts = [[xtp.tile([128, 1024], bf16, tag=f"xt{k}_{m}", name=f"xt{k}_{m}")
                for m in range(2)] for k in range(6)]

        # ---- DMA emission order == arrival priority ----
        nc.sync.dma_start(out=btA_sb, in_=btA)
        nc.sync.dma_start(out=btB_sb, in_=btB)
        nc.sync.dma_start(out=btC_sb, in_=btC)
        nc.sync.dma_start(out=btD_sb, in_=btD)
        nc.sync.dma_start(out=bvb_sb, in_=bvb)
        nc.sync.dma_start(out=triu_sb, in_=triu)
        nc.sync.dma_start(out=ones_row, in_=onesd)
        for k in range(6):
            nc.sync.dma_start(out=w1_sb[k], in_=w1[k * 128:(k + 1) * 128, :])
            nc.sync.dma_start(out=w2_sb[k], in_=w2[k * 128:(k + 1) * 128, :])
            nc.sync.dma_start(out=wv_sb[k], in_=wv[k * 128:(k + 1) * 128, :])
            nc.sync.dma_start(out=xts[k][0],
                              in_=xT[k * 128:(k + 1) * 128, 0:1024])
        for k in range(6):
            nc.sync.dma_start(out=xts[k][1],
                              in_=xT[k * 128:(k + 1) * 128, 1024:2048])
        for h in range(NHC):
            nc.sync.dma_start(out=wo_sb[h], in_=wo[64 * h:64 * (h + 1), :])

        # ---- long-lived activation tiles ----
        tileA = qkv.tile([128, S], f32r, tag="tileA")  # [qT_h0|qT_h1], q-permuted
        tileB = qkv.tile([128, S], f32r, tag="tileB")  # [kT_h0|kT_h1], natural
        tileC = qkv.tile([64, S], f32r, tag="tileC")   # qT_h2, permuted
        tileD = qkv.tile([64, S], f32r, tag="tileD")   # kT_h2, natural
        v_sb = [qkv.tile([128, 196], bf16, tag=f"v{i}", name=f"vsb{i}")
                for i in range(NT)]
        attnT = [qkv.tile([64, S], f32r, tag=f"attnT{h}", name=f"attnT{h}")
                 for h in range(NHC)]

        def permuted_copy(dst, rows, ps, n, bias):
            """psum 512-span n -> dst cols with even/odd tile permutation."""
            pr3 = ps[0:rows, :].rearrange("p (c two k) -> p c two k", two=2, k=128)
            dr = dst[0:rows, :]
            nc.vector.tensor_scalar_add(
                out=dr[:, 256 * n:256 * n + 256].rearrange("p (c k) -> p c k", k=128),
                in0=pr3[:, :, 0, :], scalar1=bias[0:rows, :])
            nc.vector.tensor_scalar_add(
                out=dr[:, 1024 + 256 * n:1024 + 256 * n + 256].rearrange(
                    "p (c k) -> p c k", k=128),
                in0=pr3[:, :, 1, :], scalar1=bias[0:rows, :])

        # ---- stage A: projections, V interleaved per span ----
        with tc.tile_pool(name="psA", bufs=2, space="PSUM") as psA, \
             tc.tile_pool(name="psV", bufs=2, space="PSUM") as psV:
            for n in range(4):
                xn = [xts[k][n // 2][:, 512 * (n % 2):512 * (n % 2) + 512]
                      for k in range(6)]
                psa = psA.tile([128, 512], f32, tag="psA")
                for k in range(6):
                    nc.tensor.matmul(psa, w1_sb[k][:, 0:128], xn[k],
                                     start=(k == 0), stop=(k == 5))
                permuted_copy(tileA, 128, psa, n, btA_sb)
                psb = psA.tile([128, 512], f32, tag="psA")
                for k in range(6):
                    nc.tensor.matmul(psb, w1_sb[k][:, 128:256], xn[k],
                                     start=(k == 0), stop=(k == 5))
                nc.vector.tensor_scalar_add(
                    out=tileB[:, 512 * n:512 * (n + 1)], in0=psb, scalar1=btB_sb)
                psq = psA.tile([64, 512], f32, tag="psq")
                psk = psA.tile([64, 512], f32, tag="psq")
                for k in range(6):
                    nc.tensor.matmul(psq, w2_sb[k][:, 0:64], xn[k],
                                     start=(k == 0), stop=(k == 5))
                    nc.tensor.matmul(psk, w2_sb[k][:, 64:128], xn[k],
                                     start=(k == 0), stop=(k == 5))
                permuted_copy(tileC, 64, psq, n, btC_sb)
                nc.vector.tensor_scalar_add(
                    out=tileD[:, 512 * n:512 * (n + 1)], in0=psk, scalar1=btD_sb)
                # V projection for the 4 s-tiles of this span
                for j in range(4):
                    st = 4 * n + j
                    psv = psV.tile([128, 192], f32, tag="psv")
                    for k in range(6):
                        nc.tensor.matmul(
                            psv, xn[k][:, 128 * j:128 * (j + 1)],
                            wv_sb[k], start=(k == 0), stop=(k == 5))
                    vt = v_sb[st]
                    nc.vector.tensor_tensor(
                        out=vt[:, 0:195].rearrange("p (h c) -> p h c", c=65)[:, :, 0:64],
                        in0=psv.rearrange("p (h c) -> p h c", c=64),
                        in1=bvb_sb.rearrange("p (h c) -> p h c", c=64), op=add)
                    nc.vector.tensor_copy(
                        out=vt[:, 0:195].rearrange("p (h c) -> p h c", c=65)[:, :, 64:65],
                        in_=ones_row[:, 0:3].unsqueeze(2))

        xtp_cm.__exit__(None, None, None)

        # ---- stage B + C: one global pipelined chain ----
        heads = [
            dict(q=(tileA, 0), k=(tileB, 0)),
            dict(q=(tileA, 64), k=(tileB, 64)),
            dict(q=(tileC, 0), k=(tileD, 0)),
        ]
        triu_b = triu_sb.unsqueeze(1).broadcast_to([128, 8, 128])

        units = [(0, 0), (1, 0), (2, 0), (0, 1), (1, 1), (2, 1)]
        flat = []  # (ui, h, grp, w, half, masked, c, C)
        for ui, (h, grp) in enumerate(units):
            if grp == 0:
                chs = [(w, 0, True) for w in range(NW)]
            else:
                chs = [(w, half, half == 1) for w in range(NW) for half in (0, 1)]
            for c, (w, half, masked) in enumerate(chs):
                flat.append((ui, h, grp, w, half, masked, c, len(chs)))
        n_flat = len(flat)  # 72

        with tc.tile_pool(name="sc", bufs=2, space="PSUM") as sc_pool, \
             tc.tile_pool(name="po", bufs=2, space="PSUM") as po_pool:

            po_of = {}
            first_of = {}
            raw_of = {}
            pts = {}
            inject = {}

            def add_inject(slot, fn):
                inject.setdefault(slot, []).append(fn)

            def emit_scores(t):
                ui, h, grp, w, half, masked, c, C = flat[t]
                if c == 0:
                    po_of[ui] = po_pool.tile([128, 1024], f32, tag="po",
                                             name=f"po{ui}")
                    first_of[ui] = [True, True]
                qt, qoff = heads[h]["q"]
                kt, koff = heads[h]["k"]
                qv = qt[qoff:qoff + 64, 1024 * grp:1024 * (grp + 1)]
                kv = kt[koff:koff + 64, :]
                kblk = kv[:, WIN * w + 128 * half:WIN * w + 128 * half + 128]
                sc = sc_pool.tile([128, 1024], f32, tag="sc")
                for sub in range(2):
                    nc.tensor.matmul(
                        sc[:, 512 * sub:512 * (sub + 1)], kblk,
                        qv[:, 512 * sub:512 * (sub + 1)],
                        start=True, stop=True)
                pt = pt_pool.tile([128, 1024], bf16, tag="pt")
                nc.scalar.activation(out=pt, in_=sc, func=Exp, scale=0.125)
                if masked:
                    p3 = pt.rearrange("p (c k) -> p c k", k=128)
                    nc.vector.tensor_mul(out=p3, in0=p3, in1=triu_b)
                pts[t] = pt

            def emit_pv(t):
                """Returns ui if this was the unit's last chunk."""
                ui, h, grp, w, half, masked, c, C = flat[t]
                po = po_of[ui]
                first = first_of[ui]
                vsl = v_sb[2 * w + half][:, 65 * h:65 * h + 65]
                pt = pts.pop(t)
                for sub in range(2):
                    nc.tensor.matmul(
                        po[0:65, 512 * sub:512 * (sub + 1)],
                        vsl, pt[:, 512 * sub:512 * (sub + 1)],
                        start=first[sub], stop=(c == C - 1))
                    first[sub] = False
                return ui if c == C - 1 else None

            def make_norm(ui, sliced=False):
                h, grp = units[ui]
                raw = raw_of[ui]
                po = po_of[ui]

                def emit_norm():
                    # broadcast den into the dead po accumulator, then recip
                    for sub in range(2):
                        nc.tensor.matmul(
                            po[0:64, 512 * sub:512 * (sub + 1)],
                            ones_row[64:65, :],
                            raw[64:65, 512 * sub:512 * (sub + 1)],
                            start=True, stop=True)
                    rec = nrm_pool.tile([64, 1024], f32, tag="rec")
                    nc.vector.reciprocal_approx_fast(out=rec, in_=po[0:64, :])
                    if not sliced:
                        nc.gpsimd.tensor_tensor(
                            out=attnT[h][:, 1024 * grp:1024 * (grp + 1)],
                            in0=raw[0:64, :], in1=rec, op=mult)
                    else:
                        # per-128-col slices, each unblocking one O-proj tile
                        for i in range(8):
                            nc.vector.tensor_tensor(
                                out=attnT[h][:, 1024 * grp + 128 * i:
                                             1024 * grp + 128 * (i + 1)],
                                in0=raw[0:64, 128 * i:128 * (i + 1)],
                                in1=rec[:, 128 * i:128 * (i + 1)], op=mult)
                            emit_stage_c(8 * grp + i)
                return emit_norm

            def emit_stage_c(p):
                pso = sc_pool.tile([128, D], f32, tag="sc")
                for (n0, n1) in ((0, 512), (512, 768)):
                    for h in range(NHC):
                        nc.tensor.matmul(
                            pso[:, n0:n1],
                            attnT[h][:, 128 * p:128 * (p + 1)],
                            wo_sb[h][:, n0:n1],
                            start=(h == 0), stop=(h == NHC - 1))
                ot = ost_pool.tile([128, D], f32, tag="ot")
                nc.vector.tensor_copy(out=ot, in_=pso)
                t = 2 * p if p < 8 else 2 * (p - 8) + 1
                nc.sync.dma_start(out=out[128 * t:128 * (t + 1), :], in_=ot)

            for s in range(n_flat + LAG):
                if s < n_flat:
                    emit_scores(s)
                for fn in inject.pop(s, []):
                    fn()
                if s >= LAG:
                    ui_done = emit_pv(s - LAG)
                    if ui_done is not None:
                        raw = nrm_pool.tile([65, 1024], f32r, tag="raw")
                        nc.vector.tensor_copy(out=raw, in_=po_of[ui_done][0:65, :])
                        raw_of[ui_done] = raw
                        add_inject(s + 2, make_norm(ui_done,
                                                    sliced=(ui_done == 5)))
                        if ui_done == 2:
                            # grp0 O-proj tiles as PE filler through grp1
                            for i in range(8):
                                add_inject(s + 4 + 5 * i,
                                           (lambda p: lambda: emit_stage_c(p))(i))
            for slot in sorted(inject):
                for fn in inject[slot]:
                    fn()

    nc.compile()
    return nc


def _prep_core_inputs(inputs, c):
    import ml_dtypes
    bf16 = ml_dtypes.bfloat16
    x = inputs["x"]
    Wq, bq = inputs["Wq"], inputs["bq"]
    Wk, bk = inputs["Wk"], inputs["bk"]
    Wv, bv = inputs["Wv"], inputs["bv"]
    Wo = inputs["Wo"]
    b = c // 4
    r0 = (c % 4) * DH  # first feature row of this core's 192-row head block

    xT = np.ascontiguousarray(np.asarray(x[b]).T.astype(bf16))
    W1 = np.ascontiguousarray(np.concatenate(
        [Wq[r0:r0 + 128].T, Wk[r0:r0 + 128].T], axis=1).astype(bf16))
    W2 = np.ascontiguousarray(np.concatenate(
        [Wq[r0 + 128:r0 + 192].T, Wk[r0 + 128:r0 + 192].T], axis=1).astype(bf16))
    Wvp = np.ascontiguousarray(Wv[r0:r0 + 192].T.astype(bf16))
    wo = np.ascontiguousarray(Wo[:, r0:r0 + 192].T.astype(np.float32))

    return dict(
        xT=xT, w1=W1, w2=W2, wv=Wvp, wo=wo,
        btA=np.ascontiguousarray(bq[r0:r0 + 128].reshape(128, 1).astype(np.float32)),
        btB=np.ascontiguousarray(bk[r0:r0 + 128].reshape(128, 1).astype(np.float32)),
        btC=np.ascontiguousarray(bq[r0 + 128:r0 + 192].reshape(64, 1).astype(np.float32)),
        btD=np.ascontiguousarray(bk[r0 + 128:r0 + 192].reshape(64, 1).astype(np.float32)),
        bvb=np.ascontiguousarray(np.tile(
            bv[r0:r0 + 192].reshape(1, 192), (128, 1)).astype(np.float32)),
        triu=np.ascontiguousarray(np.triu(np.ones((128, 128))).astype(bf16)),
        onesd=np.ones((128, 64), np.float32),
    )


def _install_ntff_hook():
    """Register antenv.axon_hooks with a ctypes NTFF profile hook so
    run_bass_kernel_spmd(trace=True) can capture device-side exec time."""
    import types, ctypes, contextlib, importlib

    try:
        import antenv.axon_hooks  # noqa: F401
        return
    except ImportError:
        pass
    so_path = "/opt/axon/libaxon_pjrt.so"
    lib = ctypes.CDLL(so_path)
    if not hasattr(lib, "axon_start_nrt_profile"):
        return
    lib.axon_start_nrt_profile.argtypes = [
        ctypes.POINTER(ctypes.c_int64), ctypes.c_size_t]
    lib.axon_start_nrt_profile.restype = ctypes.c_int64
    lib.axon_stop_nrt_profile.argtypes = [ctypes.c_char_p]
    lib.axon_stop_nrt_profile.restype = ctypes.c_int64

    @contextlib.contextmanager
    def _hook(output_dir, device_ids):
        import jax
        jax.devices()
        if device_ids:
            ids = (ctypes.c_int64 * len(device_ids))(*device_ids)
            rc = lib.axon_start_nrt_profile(ids, len(device_ids))
        else:
            rc = lib.axon_start_nrt_profile(None, 0)
        if rc != 0:
            raise RuntimeError(f"axon_start_nrt_profile rc={rc}")
        try:
            yield
        finally:
            n = lib.axon_stop_nrt_profile(str(output_dir).encode())
            print(f"profile: {n} file(s) written to {output_dir}")

    mod = types.ModuleType("antenv.axon_hooks")
    mod.get_axon_ntff_profile_hook = lambda: _hook
    mod.set_axon_ntff_profile_hook = lambda h: None
    sys.modules["antenv.axon_hooks"] = mod
    import antenv
    antenv.axon_hooks = mod


def kernel(**inputs):
    import os
    from concourse import bass_utils

    if "nc" not in _CACHE:
        _CACHE["nc"] = _build_program()
    nc = _CACHE["nc"]

    trace = bool(os.environ.get("MHA_TRACE"))
    kwargs = {}
    if trace:
        import shutil
        _install_ntff_hook()
        kwargs = dict(trace=True, tmpdir="/tmp/mha_trace")
        shutil.rmtree("/tmp/mha_trace", ignore_errors=True)
        os.makedirs("/tmp/mha_trace", exist_ok=True)

    in_maps = [_prep_core_inputs(inputs, c) for c in range(8)]
    res = bass_utils.run_bass_kernel_spmd(
        nc, in_maps, core_ids=list(range(8)), **kwargs)
    _CACHE["last_results"] = res
    if trace and res.exec_time_ns is not None:
        print(f"HW exec time: {res.exec_time_ns} ns")
    out = np.zeros((B, S, D), np.float32)
    for c in range(8):
        out[c // 4] += res.results[c]["out"]
    out += np.asarray(inputs["bo"], np.float32).reshape(1, 1, D)
    return out
